# revision 2
# baseline (speedup 1.0000x reference)
"""Trainium2 Bass kernel: 2-layer Chebyshev graph conv (K=5) + 3-layer MLP.

Distribution over 8 NeuronCores (vs v1 baseline):
  - adjacency aT shard [8192, 1024], pre-scaled by 2 on the host, is
    SBUF-RESIDENT in bf16 (16 MiB), loaded once in the preamble
    (amortized across passes like weights) instead of being streamed
    from HBM 8x per pass (256 MiB f32).
  - the Chebyshev recursion T_k = 2a T_{k-1} - T_{k-2}: the m-chunk
    sweep accumulates (2a)@T_{k-1} in PSUM; the combine is an in-place
    DVE subtract over the T_{k-2} parity tile set (T_1 = psum * 0.5
    undoes the pre-scale).
  - all graph/T-state streams, collectives and fc1 weights are bf16;
    psum and layer accumulators stay f32.
  - the per-k AllGather is split into two width halves on separate DMA
    queues; each half's spill+gather launches right after its combine,
    so a gather hides under the other half's matmul sweep.
  - fc1 contraction(row)-sharded over nodes; partial [16,512] AllReduced;
    fc2/fc3/softmax run redundantly per core.
"""

import os
import sys

import numpy as np

for _p in ("/opt/trn_rl_repo", "/root/.axon_site/_ro/trn_rl_repo"):
    if os.path.isdir(_p) and _p not in sys.path:
        sys.path.insert(0, _p)

P = 128          # SBUF partitions
N = 8192         # nodes
B = 16           # batch
F_IN = 2
F1 = 32
F2 = 32
K = 5            # Chebyshev order
NCORES = 8
R = N // NCORES  # nodes per core (1024)
MC = N // P      # m-chunks (64)
NJ = R // P      # local n-chunks (8)
C1 = 64          # padded conv1 state width (real = B*F_IN = 32)
C2 = B * F1      # 512
M1, M2, M3 = 512, 128, 2

REPEAT = int(os.environ.get("KER_REPEAT", "1"))

_CACHE = {}


def build_kernel(repeat=None):
    from concourse import bacc, mybir, tile
    from concourse.masks import make_identity

    REP = repeat if repeat is not None else REPEAT

    dt = mybir.dt
    f32 = dt.float32
    f32r = dt.float32r
    bf16 = dt.bfloat16
    Alu = mybir.AluOpType
    Act = mybir.ActivationFunctionType
    RG = [list(range(NCORES))]

    nc = bacc.Bacc(
        "TRN2",
        target_bir_lowering=False,
        debug=False,
        enable_asserts=False,
        num_devices=NCORES,
    )

    # ------------------------- DRAM I/O -------------------------
    at_d = nc.dram_tensor("at", [N, R], bf16, kind="ExternalInput").ap()
    x2dp_d = nc.dram_tensor("x2dp", [N, C1], bf16, kind="ExternalInput").ap()
    xlocT_d = nc.dram_tensor("xlocT", [C1, NJ * P], bf16, kind="ExternalInput").ap()
    w1bd_d = nc.dram_tensor("w1bd", [K, C1, C2], bf16, kind="ExternalInput").ap()
    w2bd_d = nc.dram_tensor("w2bd", [K, P, P], bf16, kind="ExternalInput").ap()
    fw1s_d = nc.dram_tensor("fw1s", [R * F2, M1], bf16, kind="ExternalInput").ap()
    fw2_d = nc.dram_tensor("fw2", [M1, M2], f32r, kind="ExternalInput").ap()
    fw3_d = nc.dram_tensor("fw3", [M2, M3], f32r, kind="ExternalInput").ap()
    b1r_d = nc.dram_tensor("b1r", [P, C2], f32, kind="ExternalInput").ap()
    b2r_d = nc.dram_tensor("b2r", [P, C2], f32, kind="ExternalInput").ap()
    fb1r_d = nc.dram_tensor("fb1r", [B, M1], f32, kind="ExternalInput").ap()
    fb2r_d = nc.dram_tensor("fb2r", [B, M2], f32, kind="ExternalInput").ap()
    fb3r_d = nc.dram_tensor("fb3r", [B, M3], f32, kind="ExternalInput").ap()
    out_d = nc.dram_tensor("out", [B, M3], f32, kind="ExternalOutput").ap()

    with tile.TileContext(nc) as tc:
        with (
            tc.tile_pool(name="consts", bufs=1) as consts,
            tc.tile_pool(name="atgp", bufs=1) as atgp,
            tc.tile_pool(name="tf1p", bufs=1) as tf1p,
            tc.tile_pool(name="ttp", bufs=3) as ttp,
            tc.tile_pool(name="stgp", bufs=1) as stgp,
            tc.tile_pool(name="accp", bufs=8) as accp,
            tc.tile_pool(name="thp", bufs=16) as thp,
            tc.tile_pool(name="tftp", bufs=4) as tftp,
            tc.tile_pool(name="ttcp", bufs=4) as ttcp,
            tc.tile_pool(name="fwp", bufs=4) as fwp,
            tc.tile_pool(name="fcp", bufs=1) as fcp,
            tc.tile_pool(name="psum", bufs=8, space="PSUM") as psp,
            tc.tile_pool(name="dram", bufs=2, space="DRAM") as drp,
        ):
            # ------------------------- constants -------------------------
            ident = consts.tile([P, P], f32)
            make_identity(nc, ident)
            identb = consts.tile([P, P], bf16)
            nc.vector.tensor_copy(identb[:], ident[:])
            identn = consts.tile([P, P], bf16)
            nc.vector.tensor_scalar_mul(identn[:], identb[:], -1.0)
            # resident adjacency shard (2x pre-scaled):
            # atg[mc][p, n] = 2*a[r0+n, 128*mc+p]
            atg = []
            for mc in range(MC):
                t = atgp.tile([P, R], bf16, name=f"atg_{mc}")
                eng = nc.sync if mc % 2 == 0 else nc.scalar
                eng.dma_start(out=t[:], in_=at_d[mc * P:(mc + 1) * P, :])
                atg.append(t)
            w1 = consts.tile([C1, K * C2], bf16)
            for k in range(K):
                nc.sync.dma_start(out=w1[:, k * C2:(k + 1) * C2], in_=w1bd_d[k])
            w2 = consts.tile([P, K * P], bf16)
            for k in range(K):
                nc.sync.dma_start(out=w2[:, k * P:(k + 1) * P], in_=w2bd_d[k])
            b1r = consts.tile([P, C2], f32)
            nc.sync.dma_start(out=b1r[:], in_=b1r_d[:])
            b2r = consts.tile([P, C2], f32)
            nc.sync.dma_start(out=b2r[:], in_=b2r_d[:])
            fb1r = consts.tile([B, M1], f32)
            nc.sync.dma_start(out=fb1r[:], in_=fb1r_d[:])
            fb2r = consts.tile([B, M2], f32)
            nc.sync.dma_start(out=fb2r[:], in_=fb2r_d[:])
            fb3r = consts.tile([B, M3], f32)
            nc.sync.dma_start(out=fb3r[:], in_=fb3r_d[:])
            fw2sb = consts.tile([P, 4 * M2], f32r)
            for c in range(4):
                nc.sync.dma_start(
                    out=fw2sb[:, c * M2:(c + 1) * M2],
                    in_=fw2_d[c * P:(c + 1) * P, :],
                )
            fw3sb = consts.tile([M2, M3], f32r)
            nc.sync.dma_start(out=fw3sb[:], in_=fw3_d[:])

            def emit_body():
                # =========================================================
                # conv1 — T^T form: local state ttr_k [C1, R] bf16, resident
                # 2a as the moving operand, recursion in PSUM.
                # =========================================================
                out1 = []
                for j in range(NJ):
                    t = accp.tile([P, C2], f32, tag="acc", name=f"out1_{j}")
                    nc.vector.memset(t[:], 0.0)
                    out1.append(t)

                def conv1_feature(ttr, k):
                    """out1[j] += (T_k^T chunk j).T @ w1[k]"""
                    for j in range(NJ):
                        f_ps = psp.tile([P, C2], f32, tag="ps", name=f"c1f_{k}_{j}")
                        nc.tensor.matmul(
                            f_ps[:],
                            ttr[:, j * P:(j + 1) * P],
                            w1[:, k * C2:(k + 1) * C2],
                            start=True,
                            stop=True,
                        )
                        nc.vector.tensor_add(out1[j][:], out1[j][:], f_ps[:])

                # T0 = x
                tf1 = tf1p.tile([P, MC * C1], bf16, tag="tf1", name="tf1_t0")
                nc.scalar.dma_start(
                    out=tf1.rearrange("p (mc f) -> p mc f", f=C1),
                    in_=x2dp_d.rearrange("(mc p) f -> p mc f", p=P),
                )
                tth = {}
                tth[0] = ttp.tile([C1, NJ * P], bf16, tag="ttr", name="c1ttr_0")
                nc.sync.dma_start(out=tth[0][:], in_=xlocT_d[:])
                conv1_feature(tth[0], 0)

                H2 = R // 2
                for k in range(1, K):
                    ps1 = [
                        psp.tile([C1, H2], f32, tag="ps", name=f"c1g_{k}_{h}")
                        for h in range(2)
                    ]
                    for mc in range(MC):
                        lhsT = tf1[:, mc * C1:(mc + 1) * C1]
                        for h in range(2):
                            nc.tensor.matmul(
                                ps1[h][:],
                                lhsT,
                                atg[mc][:, h * H2:(h + 1) * H2],
                                start=(mc == 0),
                                stop=(k == 1 and mc == MC - 1),
                            )
                    if k >= 2:
                        # psum += -T_{k-2}^T
                        for h in range(2):
                            nc.tensor.matmul(
                                ps1[h][:],
                                identn[0:C1, 0:C1],
                                tth[k - 2][:, h * H2:(h + 1) * H2],
                                start=False,
                                stop=True,
                            )
                    ttr = ttp.tile([C1, NJ * P], bf16, tag="ttr", name=f"c1ttr_{k}")
                    tth[k] = ttr
                    for h in range(2):
                        sl = ttr[:, h * H2:(h + 1) * H2]
                        if k == 1:
                            nc.vector.tensor_scalar_mul(sl, ps1[h][:], 0.5)
                        else:
                            nc.vector.tensor_copy(sl, ps1[h][:])
                    if k < K - 1:
                        # node-major staging for the AllGather
                        stg = stgp.tile([P, NJ * C1], bf16, tag="stg",
                                        name=f"c1stg_{k}")
                        for j in range(NJ):
                            st_ps = psp.tile([P, C1], bf16, tag="ps",
                                             name=f"c1st_{k}_{j}")
                            nc.tensor.transpose(
                                st_ps[:], ttr[:, j * P:(j + 1) * P],
                                identb[0:C1, 0:C1],
                            )
                            nc.vector.tensor_copy(
                                stg[:, j * C1:(j + 1) * C1], st_ps[:]
                            )
                        cc_in = drp.tile([R, C1], bf16, tag="cc1i",
                                         name=f"cc1i_{k}")
                        nc.sync.dma_start(
                            out=cc_in.rearrange("(j p) f -> p j f", p=P),
                            in_=stg.rearrange("p (j f) -> p j f", f=C1),
                        )
                        cc_out = drp.tile([N, C1], bf16, tag="cc1o",
                                          addr_space="Shared", name=f"cc1o_{k}")
                        nc.gpsimd.collective_compute(
                            "AllGather", Alu.bypass, replica_groups=RG,
                            ins=[cc_in.opt()], outs=[cc_out.opt()],
                        )
                        tf1 = tf1p.tile([P, MC * C1], bf16, tag="tf1",
                                        name=f"tf1_{k}")
                        nc.scalar.dma_start(
                            out=tf1.rearrange("p (mc f) -> p mc f", f=C1),
                            in_=cc_out.rearrange("(mc p) f -> p mc f", p=P),
                        )
                    conv1_feature(ttr, k)

                # conv1 epilogue: h = relu(out1 + b1) -> bf16 T0 tiles + gather
                th0 = []
                for j in range(NJ):
                    nc.vector.tensor_add(out1[j][:], out1[j][:], b1r[:])
                    t = thp.tile([P, C2], bf16, tag="th", name=f"th0_{j}")
                    nc.vector.tensor_relu(t[:], out1[j][:])
                    th0.append(t)
                # width-split (SG) gathers so the collective pipelines with
                # the next sweep
                SG = 2
                W = C2 // SG
                cc_h = []
                for s in range(SG):
                    cc_hi = drp.tile([R, W], bf16, tag=f"cc2i{s}",
                                     name=f"cc_hi_{s}")
                    for j in range(NJ):
                        nc.sync.dma_start(
                            out=cc_hi[j * P:(j + 1) * P, :],
                            in_=th0[j][:, s * W:(s + 1) * W],
                        )
                    cc_o = drp.tile([N, W], bf16, tag=f"cc2o{s}",
                                    addr_space="Shared", name=f"cc_h_{s}")
                    nc.gpsimd.collective_compute(
                        "AllGather", Alu.bypass, replica_groups=RG,
                        ins=[cc_hi.opt()], outs=[cc_o.opt()],
                    )
                    cc_h.append(cc_o)

                # =========================================================
                # conv2 — node-major, resident 2a slices stationary,
                # recursion in PSUM.
                # =========================================================
                out2 = []
                for j in range(NJ):
                    t = accp.tile([P, C2], f32, tag="acc", name=f"out2_{j}")
                    nc.vector.memset(t[:], 0.0)
                    out2.append(t)

                def conv2_feature(tk, k):
                    """out2[j] += sum_c transpose(tk[j] colblock c).T @ w2[k]"""
                    for j in range(NJ):
                        f_ps = psp.tile([P, C2], f32, tag="ps",
                                        name=f"c2f_{k}_{j}")
                        for c in range(4):
                            tt_ps = psp.tile([P, P], bf16, tag="ps",
                                             name=f"c2tp_{k}_{j}_{c}")
                            nc.tensor.transpose(
                                tt_ps[:], tk[j][:, c * P:(c + 1) * P], identb[:]
                            )
                            tt = ttcp.tile([P, P], bf16, tag="tt",
                                           name=f"c2tt_{k}_{j}_{c}")
                            nc.vector.tensor_copy(tt[:], tt_ps[:])
                            nc.tensor.matmul(
                                f_ps[:, c * P:(c + 1) * P],
                                tt[:],
                                w2[:, k * P:(k + 1) * P],
                                start=True,
                                stop=True,
                            )
                        nc.vector.tensor_add(out2[j][:], out2[j][:], f_ps[:])

                conv2_feature(th0, 0)

                # parity tile sets: thE (=th0) holds T0/T2/T4 in place, thO
                # holds T1/T3 in place — the recursion combine is an in-place
                # DVE subtract (T_k = psum - T_{k-2}), no extra generations.
                thO = [
                    thp.tile([P, C2], bf16, tag="th", name=f"thO_{j}")
                    for j in range(NJ)
                ]
                src = cc_h
                for k in range(1, K):
                    dst = thO if k % 2 == 1 else th0
                    ps_g = [
                        psp.tile([P, C2], f32, tag="ps", name=f"c2g_{k}_{j}")
                        for j in range(NJ)
                    ]
                    nxt = [] if k < K - 1 else None
                    for s in range(SG):
                        # per-s queues: s=0 on sync, s=1 on scalar, so a spill
                        # waiting on the s-combine never blocks the other
                        # half's tft stream (FIFO queues)
                        q = nc.sync if s == 0 else nc.scalar
                        sl = slice(s * W, (s + 1) * W)
                        for mc in range(MC):
                            tft = tftp.tile([P, W], bf16, tag="tft",
                                            name=f"tf2_{k}_{s}_{mc}")
                            q.dma_start(
                                out=tft[:], in_=src[s][mc * P:(mc + 1) * P, :]
                            )
                            for j in range(NJ):
                                nc.tensor.matmul(
                                    ps_g[j][:, sl],
                                    atg[mc][:, j * P:(j + 1) * P],
                                    tft[:],
                                    start=(mc == 0),
                                    stop=(mc == MC - 1),
                                )
                        for j in range(NJ):
                            if k == 1:
                                nc.vector.tensor_scalar_mul(
                                    dst[j][:, sl], ps_g[j][:, sl], 0.5
                                )
                            else:
                                # T_k = psum - T_{k-2}, in place over T_{k-2}
                                nc.vector.scalar_tensor_tensor(
                                    dst[j][:, sl], ps_g[j][:, sl], 1.0,
                                    dst[j][:, sl], Alu.mult, Alu.subtract,
                                )
                        if k < K - 1:
                            cc_in = drp.tile([R, W], bf16, tag=f"cc2i{s}",
                                             name=f"cc2i_{k}_{s}")
                            for j in range(NJ):
                                q.dma_start(
                                    out=cc_in[j * P:(j + 1) * P, :],
                                    in_=dst[j][:, sl],
                                )
                            cc_out = drp.tile([N, W], bf16, tag=f"cc2o{s}",
                                              addr_space="Shared",
                                              name=f"cc2o_{k}_{s}")
                            nc.gpsimd.collective_compute(
                                "AllGather", Alu.bypass, replica_groups=RG,
                                ins=[cc_in.opt()], outs=[cc_out.opt()],
                            )
                            nxt.append(cc_out)
                    if k < K - 1:
                        src = nxt
                    conv2_feature(dst, k)

                # conv2 epilogue: out2 = relu(out2 + b2) -> bf16 for fc1
                # (reuses the thO tiles: T3 has no readers left)
                fc_lhs = thO
                for j in range(NJ):
                    nc.vector.tensor_add(out2[j][:], out2[j][:], b2r[:])
                    nc.vector.tensor_relu(thO[j][:], out2[j][:])

                # =========================================================
                # fc1 (node-sharded contraction) + AllReduce
                # =========================================================
                fw1v = fw1s_d.rearrange("(j p f) m -> j f p m", p=P, f=F2)
                fc_ps = psp.tile([B, M1], f32, tag="ps", name="fc1_ps")
                n_mm = NJ * F2
                i_mm = 0
                for j in range(NJ):
                    lhs_j = fc_lhs[j].rearrange("p (b f) -> p f b", f=F2)
                    for f in range(F2):
                        fwt = fwp.tile([P, M1], bf16, tag="fw",
                                       name=f"fw1_{j}_{f}")
                        feng = nc.scalar if f % 2 == 0 else nc.sync
                        feng.dma_start(out=fwt[:], in_=fw1v[j, f])
                        nc.tensor.matmul(
                            fc_ps[:],
                            lhs_j[:, f, :],
                            fwt[:],
                            start=(i_mm == 0),
                            stop=(i_mm == n_mm - 1),
                        )
                        i_mm += 1

                z = fcp.tile([B, M1], f32)
                nc.vector.tensor_copy(z[:], fc_ps[:])
                cc_fi = drp.tile([B, M1], f32, tag="ccfi")
                nc.sync.dma_start(out=cc_fi[:], in_=z[:])
                cc_fo = drp.tile([B, M1], f32, tag="ccfo", addr_space="Shared")
                nc.gpsimd.collective_compute(
                    "AllReduce", Alu.add, replica_groups=RG,
                    ins=[cc_fi.opt()], outs=[cc_fo.opt()],
                )

                # z1 = relu(fc1 + fb1), padded to 32 partitions for PE transpose
                z1p = fcp.tile([32, M1], f32)
                nc.vector.memset(z1p[:], 0.0)
                nc.sync.dma_start(out=z1p[0:B, :], in_=cc_fo[:])
                nc.vector.tensor_add(z1p[0:B, :], z1p[0:B, :], fb1r[:])
                nc.vector.tensor_relu(z1p[0:B, :], z1p[0:B, :])

                # fc2
                fc2_ps = psp.tile([B, M2], f32, tag="ps", name="fc2_ps")
                for c in range(4):
                    zt_ps = psp.tile([P, 32], f32, tag="ps", name=f"ztps_{c}")
                    nc.tensor.transpose(
                        zt_ps[:], z1p[:, c * P:(c + 1) * P], ident[0:32, 0:32]
                    )
                    zt = fcp.tile([P, 32], f32r, tag="zt", name=f"zt_{c}")
                    nc.vector.tensor_copy(zt[:], zt_ps[:])
                    nc.tensor.matmul(
                        fc2_ps[:],
                        zt[:, 0:B],
                        fw2sb[:, c * M2:(c + 1) * M2],
                        start=(c == 0),
                        stop=(c == 3),
                    )
                z2p = fcp.tile([32, M2], f32)
                nc.vector.memset(z2p[:], 0.0)
                nc.vector.tensor_copy(z2p[0:B, :], fc2_ps[:])
                nc.vector.tensor_add(z2p[0:B, :], z2p[0:B, :], fb2r[:])
                nc.vector.tensor_relu(z2p[0:B, :], z2p[0:B, :])

                # fc3
                z3t_ps = psp.tile([P, 32], f32, tag="ps", name="z3tps")
                nc.tensor.transpose(z3t_ps[:], z2p[:], ident[0:32, 0:32])
                z3t = fcp.tile([P, 32], f32r)
                nc.vector.tensor_copy(z3t[:], z3t_ps[:])
                fc3_ps = psp.tile([B, M3], f32, tag="ps", name="fc3_ps")
                nc.tensor.matmul(
                    fc3_ps[:], z3t[:, 0:B], fw3sb[:], start=True, stop=True
                )
                s = fcp.tile([B, M3], f32)
                nc.vector.tensor_copy(s[:], fc3_ps[:])
                nc.vector.tensor_add(s[:], s[:], fb3r[:])

                # softmax over the last dim (M3 = 2)
                mx = fcp.tile([B, 1], f32)
                nc.vector.reduce_max(mx[:], s[:], axis=mybir.AxisListType.X)
                nc.vector.tensor_scalar_mul(mx[:], mx[:], -1.0)
                nc.scalar.activation(s[:], s[:], Act.Exp, bias=mx[:, 0:1])
                sm = fcp.tile([B, 1], f32)
                nc.vector.reduce_sum(sm[:], s[:], axis=mybir.AxisListType.X)
                nc.vector.reciprocal(sm[:], sm[:])
                nc.vector.tensor_scalar_mul(s[:], s[:], sm[:, 0:1])
                nc.sync.dma_start(out=out_d[:], in_=s[:])

            for _rep in range(REP):
                emit_body()

    nc.compile()
    return nc


def prepare_inputs(x, a, w1, b1, w2, b2, fw1, fb1, fw2, fb2, fw3, fb3):
    """Shard + re-layout the full model inputs into 8 per-core input maps."""
    import ml_dtypes

    bf = ml_dtypes.bfloat16

    x = np.asarray(x, np.float32)
    a = np.asarray(a, np.float32)
    w1 = np.asarray(w1, np.float32)
    w2 = np.asarray(w2, np.float32)
    fw1 = np.asarray(fw1, np.float32)

    # node-major [N, B*F_IN] padded to C1
    x2d = x.transpose(1, 0, 2).reshape(N, B * F_IN).astype(np.float32)
    x2dp = np.zeros((N, C1), np.float32)
    x2dp[:, : B * F_IN] = x2d
    x2dp_c = x2dp.astype(bf)

    w1bd = np.zeros((K, C1, C2), np.float32)
    for b in range(B):
        w1bd[:, b * F_IN:(b + 1) * F_IN, b * F1:(b + 1) * F1] = w1
    w2bd = np.zeros((K, P, P), np.float32)
    for q in range(4):
        w2bd[:, q * F1:(q + 1) * F1, q * F2:(q + 1) * F2] = w2

    b1r = np.broadcast_to(np.tile(np.asarray(b1, np.float32), B), (P, C2)).copy()
    b2r = np.broadcast_to(np.tile(np.asarray(b2, np.float32), B), (P, C2)).copy()
    fb1r = np.broadcast_to(np.asarray(fb1, np.float32), (B, M1)).copy()
    fb2r = np.broadcast_to(np.asarray(fb2, np.float32), (B, M2)).copy()
    fb3r = np.broadcast_to(np.asarray(fb3, np.float32), (B, M3)).copy()
    fw2_c = np.asarray(fw2, np.float32)
    fw3_c = np.asarray(fw3, np.float32)
    w1bd_c = w1bd.astype(bf)
    w2bd_c = w2bd.astype(bf)

    a2t = np.ascontiguousarray((2.0 * a).T)  # pre-scaled for in-PSUM recursion
    fw1_3 = fw1.reshape(N, F2, M1)

    in_maps = []
    for i in range(NCORES):
        r0 = i * R
        at_i = np.ascontiguousarray(a2t[:, r0:r0 + R]).astype(bf)
        xlocT = np.ascontiguousarray(x2dp[r0:r0 + R].T).astype(bf)
        fw1s = np.ascontiguousarray(
            fw1_3[r0:r0 + R].reshape(R * F2, M1)
        ).astype(bf)
        in_maps.append(
            {
                "at": at_i,
                "x2dp": x2dp_c,
                "xlocT": xlocT,
                "w1bd": w1bd_c,
                "w2bd": w2bd_c,
                "fw1s": fw1s,
                "fw2": fw2_c,
                "fw3": fw3_c,
                "b1r": b1r,
                "b2r": b2r,
                "fb1r": fb1r,
                "fb2r": fb2r,
                "fb3r": fb3r,
            }
        )
    return in_maps


def kernel(**inputs) -> np.ndarray:
    from concourse.bass_utils import run_bass_kernel_spmd

    key = ("v2",)
    if key not in _CACHE:
        _CACHE[key] = build_kernel()
    nc = _CACHE[key]

    in_maps = prepare_inputs(**inputs)
    res = run_bass_kernel_spmd(nc, in_maps, core_ids=list(range(NCORES)))
    return np.asarray(res.results[0]["out"], np.float32)
